# revision 12
# baseline (speedup 1.0000x reference)
"""Trainium2 Bass kernel for CholeskyMMNet (optimized).

Math (per sample b):
  x  = relu(q @ W_in + b_in)
  x  = relu(x @ W_h1 + b_h1) + x
  x  = relu(x @ W_h2 + b_h2) + x
  P  = x @ (W_out @ W_o) + (b_out @ W_o + b_o)   # head fused on host
  L  = tril matrix from P (diag gets +2.0)
  M  = L @ L.T                                    # [32, 32] SPD output

Distribution: pure data parallel over 8 NeuronCores (batch 65536 -> 8192/core,
16 tiles of 512 samples each, fully unrolled + software-pipelined).

Design highlights (per core):
  * all matmuls in bf16 (abs error budget is ~3.5; incurred ~1.3)
  * q pre-transposed on host -> feature-major [32, bc]; strided tile loads
  * head emits "j-major column-L" CL2[b, 32j+k] = L_b[j,k] in bf16;
    zero padding and the +2.0 diag bias are baked into W_cl / b_cl
  * ONE DVE StreamTranspose per 128 samples turns CL2 into
    CLX[32r+k, 32j+p] = L_{32r+p}[j,k] -- no shuffle DMAs at all
  * LLT: per-sample square matmul with lhsT = rhs =
    CLX[32r:32r+32, p::32] (stride-32 free AP) at tile_position
    (32r, 32c); out = M_b as a [32,32] psum block, 64 samples per bank
  * software pipeline: each tile's 512 LLT matmuls are emitted between
    the NEXT tile's MLP stages, hiding all evac latencies (PE ~95% busy);
    the first two tiles' MLPs are co-emitted stage-interleaved
  * output staged [128, 4096] bf16 per tile, DMA'd raw with 8KB
    descriptors; the (c,i),(g,h,w,j) -> (b,i,j) unscramble happens on
    the host in numpy (fp32 cast there)

Host-visible sample mapping (tile t, CL group g, u in [0,128)):
  r = u>>5, p = u&31   (CLX placement);  h = u>>6, c = (u>>4)&3, w = u&15
  m_raw[t, 32c+i, 1024g + 512h + 32w + j] = M_b[i,j],  b = 512t + 128g + u
"""

import numpy as np

QDIM = 32
H = 512
N_CORES = 8
TILE_B = 512
# packed psum offsets of head octiles 0..6 (sizes 16,32,48,64,80,96,112)
OCT_OFF = [0, 16, 48, 96, 160, 240, 336]


def _host_prep(q, W_in, b_in, W_h1, b_h1, W_h2, b_h2, W_out, b_out, W_o, b_o):
    """Fuse W_out into the head, reorder columns into CL layout, cast."""
    import ml_dtypes

    bf16 = ml_dtypes.bfloat16
    f64 = np.float64
    Wf = W_out.astype(f64) @ W_o.astype(f64)                    # [512, 528]
    bf = b_out.astype(f64) @ W_o.astype(f64) + b_o.astype(f64)  # [528]

    # j-major CL layout: CL2[b, 32j+k] = L_b[j,k]
    # param index for L[j, k]:  j == k -> j ;  j > k -> 32 + j(j-1)/2 + k
    W_cl = np.zeros((H, 1024), f64)
    b_cl = np.zeros(1024, f64)
    for j in range(QDIM):
        for k in range(j + 1):
            c = j if j == k else QDIM + (j * (j - 1)) // 2 + k
            W_cl[:, QDIM * j + k] = Wf[:, c]
            b_cl[QDIM * j + k] = bf[c] + (2.0 if j == k else 0.0)

    # head compute layout: block j only needs its k < 4*ceil((j+1)/4)
    # columns; pack per j-octile o (4 blocks of width w=4(o+1)) at psum
    # offsets OCT_OFF[o]. Octile 7 is full-width -> stays padded at 896.
    # The evac scatters packed octiles back to padded positions.
    Wc = np.zeros((H, 1024), f64)
    for off, o in zip(OCT_OFF, range(7)):
        w = 4 * (o + 1)
        for i, j in enumerate(range(4 * o, 4 * o + 4)):
            Wc[:, off + w * i : off + w * i + w] = W_cl[:, 32 * j : 32 * j + w]
    Wc[:, 896:] = W_cl[:, 896:]
    W_cl = Wc

    def chunked(W):
        """[512, n] -> [128, 4n]: chunk k (rows 128k..) at cols n*k.."""
        n = W.shape[1]
        out = np.zeros((128, 4 * n), np.float64)
        for k in range(4):
            out[:, n * k : n * (k + 1)] = W[128 * k : 128 * (k + 1)]
        return np.ascontiguousarray(out.astype(bf16))

    b3 = np.stack([b_in, b_h1, b_h2], 0).reshape(3, 4, 128)  # [l, m, c]
    prep = {
        "w_in": np.ascontiguousarray(W_in.astype(bf16)),
        "b_mlp": np.ascontiguousarray(
            b3.transpose(2, 0, 1).reshape(128, 12), np.float32
        ),
        "w_h1": chunked(W_h1),
        "w_h2": chunked(W_h2),
        "w_cl": chunked(W_cl),
        "b_cl_rep": np.ascontiguousarray(
            np.tile(b_cl.astype(bf16)[None, :], (128, 1))
        ),
    }
    return prep


def build_nc(nt, use_for_i=False, debug_taps=False):
    """Build the per-core Bass program for nt tiles of 512 samples."""
    from contextlib import ExitStack

    import concourse.bacc as bacc
    import concourse.mybir as mybir
    import concourse.tile as tile
    from concourse.ap import AP

    f32 = mybir.dt.float32
    bf16 = mybir.dt.bfloat16
    Relu = mybir.ActivationFunctionType.Relu
    bc = nt * TILE_B

    nc = bacc.Bacc(None, target_bir_lowering=False, debug=False)

    q_d = nc.dram_tensor("q_t", [QDIM, bc], bf16, kind="ExternalInput")
    w_in_d = nc.dram_tensor("w_in", [QDIM, H], bf16, kind="ExternalInput")
    b_mlp_d = nc.dram_tensor("b_mlp", [128, 12], f32, kind="ExternalInput")
    w_h1_d = nc.dram_tensor("w_h1", [128, 4 * H], bf16, kind="ExternalInput")
    w_h2_d = nc.dram_tensor("w_h2", [128, 4 * H], bf16, kind="ExternalInput")
    w_cl_d = nc.dram_tensor("w_cl", [128, 4096], bf16, kind="ExternalInput")
    b_cl_d = nc.dram_tensor("b_cl_rep", [128, 1024], bf16, kind="ExternalInput")
    m_d = nc.dram_tensor("m_raw", [nt * 128, 4096], bf16, kind="ExternalOutput")
    if debug_taps:
        dbg_x1 = nc.dram_tensor("dbg_x1", [128, TILE_B], bf16, kind="ExternalOutput")
        dbg_x3 = nc.dram_tensor("dbg_x3", [128, TILE_B], bf16, kind="ExternalOutput")
        dbg_cl = nc.dram_tensor("dbg_cl", [128, 1024], bf16, kind="ExternalOutput")
        dbg_s = nc.dram_tensor("dbg_s", [128, 1024], bf16, kind="ExternalOutput")

    def ap(t, off, dims):
        h = t.tensor if isinstance(t, AP) else t
        return AP(h, off, [[s, c] for s, c in dims])

    with tile.TileContext(nc) as tc, ExitStack() as ctx:
        const = ctx.enter_context(tc.tile_pool(name="const", bufs=1))
        qpool = ctx.enter_context(tc.tile_pool(name="qpool", bufs=2))
        xpool = ctx.enter_context(tc.tile_pool(name="xpool", bufs=2))
        clpool = ctx.enter_context(tc.tile_pool(name="clpool", bufs=2))
        spool = ctx.enter_context(tc.tile_pool(name="spool", bufs=3))
        opool = ctx.enter_context(tc.tile_pool(name="opool", bufs=3))
        ps_mlp = ctx.enter_context(tc.tile_pool(name="ps_mlp", bufs=2, space="PSUM"))
        ps_head = ctx.enter_context(tc.tile_pool(name="ps_head", bufs=2, space="PSUM"))
        ps_llt = ctx.enter_context(tc.tile_pool(name="ps_llt", bufs=2, space="PSUM"))

        # ---- prime the activation table before anything else ----
        warm = const.tile([128, 1], bf16, name="warm")
        nc.vector.memset(warm[:, :], 0.0)
        warm2 = const.tile([128, 1], bf16, name="warm2")
        nc.scalar.activation(warm2[:, :], warm[:, :], Relu)

        # ---- resident weights (biases + w_in first: L1 needs them) ----
        b_mlp_sb = const.tile([128, 12], f32, name="b_mlp_sb")
        nc.sync.dma_start(b_mlp_sb[:, :], b_mlp_d[:, :])
        w_in_sb = const.tile([QDIM, H], bf16, name="w_in_sb")
        nc.sync.dma_start(w_in_sb[:, :], w_in_d[:, :])
        b_cl_sb = const.tile([128, 1024], bf16, name="b_cl_sb")
        nc.sync.dma_start(b_cl_sb[:, :], b_cl_d[:, :])
        # big weight tiles; DMAs emitted later (after q prefetches) on SWDGE
        wh_sb = [
            const.tile([128, 4 * H], bf16, name=n, tag=n) for n in ("wh1", "wh2")
        ]
        wcl_t = const.tile([128, 4096], bf16, name="wcl", tag="wcl")

        def load_big_weights():
            nc.gpsimd.dma_start(wh_sb[0][:, :], w_h1_d[:, :])
            nc.gpsimd.dma_start(wh_sb[1][:, :], w_h2_d[:, :])
            nc.gpsimd.dma_start(wcl_t[:, :], w_cl_d[:, :])

        def llt_group(sg, o_sb, g):
            """LLT for one CL group: 128 square matmuls + 2 evacs."""
            for h in range(2):
                pl = ps_llt.tile([128, 512], f32, name="ps_llt", tag="ps_llt")
                for u in range(64 * h, 64 * h + 64):
                    r, p = u >> 5, u & 31
                    c, w = (u >> 4) & 3, u & 15
                    blk = ap(sg, 32 * r * 1024 + p, [(1024, 32), (32, 32)])
                    nc.tensor.matmul(
                        pl[32 * c : 32 * c + 32, 32 * w : 32 * w + 32],
                        blk,
                        blk,
                        start=True,
                        stop=True,
                        tile_position=(32 * r, 32 * c),
                    )
                nc.scalar.copy(
                    o_sb[:, 1024 * g + 512 * h : 1024 * g + 512 * (h + 1)],
                    pl[:, :],
                )

        from collections import deque

        pending = deque()

        def fill(budget):
            """Emit up to `budget` pending LLT groups (oldest tile first).

            Each group's [128, 1024] output slice is DMA'd as soon as its
            two psum evacs land, keeping the tail short."""
            while budget > 0 and pending:
                st = pending[0]
                g = st["next_g"]
                if g >= len(st["sg"]):
                    return  # this tile hasn't produced that group yet
                llt_group(st["sg"][g], st["o_sb"], g)
                nc.gpsimd.dma_start(
                    ap(
                        m_d,
                        st["t"] * 128 * 4096 + 1024 * g,
                        [(4096, 128), (1, 1024)],
                    ),
                    ap(st["o_sb"], 1024 * g, [(4096, 128), (1, 1024)]),
                )
                st["next_g"] += 1
                if st["next_g"] == 4:
                    pending.popleft()
                budget -= 1

        def tile_stages(toff, fills, last=False):
            """Generator emitting one tile; yields between pipeline stages.

            `fills` = LLT-group budgets (from older tiles' pending work)
            emitted after L1 / R1 / R2, filling the PE's evac-latency
            bubbles in this tile's MLP."""
            # -- q tile: host pre-transposed q_t[j, b]; plain strided load --
            q_T = qpool.tile([QDIM, TILE_B], bf16, name="q_T", tag="q_T")
            nc.gpsimd.dma_start(
                q_T[:, :],
                ap(q_d, toff, [(bc, QDIM), (1, TILE_B)]),
            )

            # -- layer 1: x1 = relu(W_in.T @ q_T + b_in) --
            x1 = []
            for m in range(4):
                ps = ps_mlp.tile([128, TILE_B], f32, name="ps1", tag="ps_mlp")
                nc.tensor.matmul(
                    ps[:, :],
                    w_in_sb[:, 128 * m : 128 * (m + 1)],
                    q_T[:, :],
                    start=True,
                    stop=True,
                )
                xm = xpool.tile([128, TILE_B], bf16, name=f"x1_{m}", tag=f"x1_{m}")
                nc.scalar.activation(
                    xm[:, :], ps[:, :], Relu, bias=b_mlp_sb[:, m : m + 1]
                )
                x1.append(xm)

            yield
            fill(fills[0])

            # -- residual blocks --
            def res_block(xin, w_t, boff, oname):
                xout = []
                for m in range(4):
                    ps = ps_mlp.tile([128, TILE_B], f32, name="psh", tag="ps_mlp")
                    for k in range(4):
                        nc.tensor.matmul(
                            ps[:, :],
                            w_t[:, H * k + 128 * m : H * k + 128 * (m + 1)],
                            xin[k][:, :],
                            start=(k == 0),
                            stop=(k == 3),
                        )
                    tmp = xpool.tile([128, TILE_B], bf16, name="tmp", tag="tmp")
                    nc.scalar.activation(
                        tmp[:, :], ps[:, :], Relu,
                        bias=b_mlp_sb[:, boff + m : boff + m + 1],
                    )
                    xm = xpool.tile(
                        [128, TILE_B], bf16, name=f"{oname}_{m}", tag=f"{oname}_{m}"
                    )
                    nc.vector.tensor_add(xm[:, :], tmp[:, :], xin[m][:, :])
                    xout.append(xm)
                return xout

            x2 = res_block(x1, wh_sb[0], 4, "x2")
            yield
            fill(fills[1])
            x3 = res_block(x2, wh_sb[1], 8, "x3")
            yield
            fill(fills[2])

            o_sb = opool.tile([128, 4096], bf16, name="o_sb", tag="o_sb")
            cur = {"t": toff // TILE_B, "sg": [], "o_sb": o_sb, "next_g": 0}
            pending.append(cur)
            for g in range(4):
                # -- head: CL_g = x3[:, g].T @ W_cl + b_cl  (batch-major) --
                psh = ps_head.tile([128, 1024], f32, name="ps_head", tag="ps_head")
                spans = [(OCT_OFF[o], 16 * (o + 1)) for o in range(7)]
                spans.append((896, 128))
                for lo, sz in spans:
                    for k in range(4):
                        nc.tensor.matmul(
                            psh[:, lo : lo + sz],
                            x3[k][:, 128 * g : 128 * (g + 1)],
                            wcl_t[:, 1024 * k + lo : 1024 * k + lo + sz],
                            start=(k == 0),
                            stop=(k == 3),
                        )
                clg = clpool.tile([128, 1024], bf16, name=f"cl_{g}", tag=f"cl_{g}")
                # per-octile: zero the k>=w tail of each block, then biased
                # evac scattering the packed psum octile to padded positions
                nc.vector.tensor_add(
                    clg[:, 896:1024], psh[:, 896:1024], b_cl_sb[:, 896:1024]
                )
                for o in range(7):
                    w = 4 * (o + 1)
                    nc.gpsimd.memset(
                        ap(clg, 128 * o + w, [(1024, 128), (32, 4), (1, 32 - w)]),
                        0.0,
                    )
                    nc.vector.tensor_add(
                        ap(clg, 128 * o, [(1024, 128), (32, 4), (1, w)]),
                        ap(psh, OCT_OFF[o], [(1024, 128), (w, 4), (1, w)]),
                        ap(b_cl_sb, 128 * o, [(1024, 128), (32, 4), (1, w)]),
                    )

                # -- block transpose on DVE (partition<->free swap per 32x32):
                #    CLX_g[32r+k, 32j+p] = CL2_g[32r+p, 32j+k] = L_{32r+p}[j,k]
                sg = spool.tile([128, 1024], bf16, name=f"s_{g}", tag=f"s_{g}")
                nc.vector.transpose(sg[:, :], clg[:, :])
                cur["sg"].append(sg)
                if last:
                    # final tile: interleave own LLT behind head groups
                    fill(1)
                if debug_taps and g == 0:
                    nc.sync.dma_start(dbg_x1[:, :], x1[0][:, :])
                    nc.sync.dma_start(dbg_x3[:, :], x3[0][:, :])
                    nc.sync.dma_start(dbg_cl[:, :], clg[:, :])
                    nc.sync.dma_start(dbg_s[:, :], sg[:, :])

        assert not use_for_i, "pipelined emission requires unrolled build"
        assert nt >= 2
        # Prologue: co-emit the first two tiles stage-interleaved so each
        # one's evac latencies are hidden by the other's matmuls.
        gens = [
            tile_stages(0, (0, 0, 0)),
            tile_stages(TILE_B, (0, 0, 0), last=(nt == 2)),
        ]
        alive = list(gens)
        first = True
        while alive:
            for gnr in list(alive):
                try:
                    next(gnr)
                except StopIteration:
                    alive.remove(gnr)
            if first:
                # q0/q1 are now ahead of these on the SWDGE queue
                load_big_weights()
                first = False
        # Steady state: each tile drains one older tile's 4 LLT groups.
        for t in range(2, nt):
            for _ in tile_stages(t * TILE_B, (2, 1, 1), last=(t == nt - 1)):
                pass
        fill(1 << 30)

    nc.compile()
    return nc


_NC_CACHE = {}


def _get_nc(nt, use_for_i=False):
    key = (nt, use_for_i)
    if key not in _NC_CACHE:
        _NC_CACHE[key] = build_nc(nt, use_for_i)
    return _NC_CACHE[key]


LAST_RESULTS = None


def _unscramble(raw, nt):
    """[nt*128, 4096] raw tile layout -> [nt*512, 32, 32] fp32."""
    r = np.asarray(raw).reshape(nt, 4, 32, 4, 2, 16, 32)
    return (
        r.transpose(0, 3, 4, 1, 5, 2, 6)
        .reshape(nt * TILE_B, QDIM, QDIM)
        .astype(np.float32)
    )


def kernel(**inputs):
    import os

    import ml_dtypes

    from concourse.bass_utils import run_bass_kernel_spmd

    global LAST_RESULTS
    q = np.asarray(inputs["q"], np.float32)
    B = q.shape[0]
    bc = B // N_CORES
    nt = bc // TILE_B
    prep = _host_prep(**{k: np.asarray(v) for k, v in inputs.items()})
    q_bf = q.astype(ml_dtypes.bfloat16)

    nc = _get_nc(nt)
    in_maps = []
    for c in range(N_CORES):
        m = dict(prep)
        m["q_t"] = np.ascontiguousarray(q_bf[c * bc : (c + 1) * bc].T)
        in_maps.append(m)
    trace = os.environ.get("KERNEL_TRACE") == "1"
    res = run_bass_kernel_spmd(
        nc, in_maps, core_ids=list(range(N_CORES)), trace=trace
    )
    LAST_RESULTS = res
    out = np.concatenate(
        [_unscramble(r["m_raw"], nt) for r in res.results], axis=0
    )
    return out
